# revision 18
# baseline (speedup 1.0000x reference)
"""Bass/Tile TRN2 kernel for nn_Attention (B=32, S=2048, D=1024), 8 cores.

Algorithm (algebraically equal to the reference):
    v[b,:]  = st[b] @ W                  (tiny matmul; avoids the huge hx@W^T)
    score   = (hx . v) * mask            (softmax shift-invariance: the b.st
                                          constant and the 1e-18 epsilon drop out)
    e       = exp(score - max); L = sum(e)
    u[b,:]  = e . hx                     (PE matmul, hx in native layout)
    ct      = (u @ W^T)/L + b            (1/L applied once at the end)

Data-parallel over batch: each of the 8 cores gets 4 batches; W/b replicated.

V2 design (fp32 / dual-HWDGE / rolling window / natural token map):
  - hx streamed fp32 over BOTH HWDGE rings (sync + scalar alternating) --
    dual-ring sustains ~410 GB/s vs ~325 single-ring and dodges the SWDGE
    engine-15 descriptor-ring straggle that cost the f16-cast version ~25us.
  - token map: partition p holds tokens 16p+i (i = column block 0..15), so
    every hx descriptor is a 16KB contiguous DRAM run and the mask DMAs
    straight into its softmax layout (no transpose). Softmax/u algebra is
    token-order agnostic; only ct's d/e dims must stay in natural order.
  - SBUF can't hold 4 batches of fp32, so a 2-batch rolling window: scores
    chase each chunk as it lands, u matmuls run right after each batch's
    softmax, freeing the buffer for batch b+2. Batch b+2's chunk DMAs are
    traced after every reader of batch b (WAR through the Tile dep graph).
  - v/u/broadcast matmuls run as float32r (1 cycle/row when moving >=256).
  - last batch's softmax is two-stage (cols 0..NA-1 / NA..15) so only the
    taper chunks' work runs after the final hx bytes land.
  - ACT-queue ordering is load-bearing: hx dma_starts are interleaved with
    the per-batch ACT compute so no long sem-wait blocks a pending issue.
"""

import numpy as np
from contextlib import ExitStack

import concourse.bass as bass
import concourse.bacc as bacc
import concourse.mybir as mybir
import concourse.tile as tile
from concourse.bass_utils import run_bass_kernel_spmd

B, S, D = 32, 2048, 1024
NCORES = 8
BPC = B // NCORES          # 4 batches per core
P = 128
NT = S // P                # 16 token-columns per batch (token = 16p + i)
DCH = D // P               # 8 chunks of 128 along D
WJ = D // P                # 8 W rows per partition (row = 8p + j)
HF = 512                   # PSUM bank limit: 512 fp32 per partition

F32 = mybir.dt.float32
F32R = mybir.dt.float32r
F16 = mybir.dt.float16
AF = mybir.ActivationFunctionType
ALU = mybir.AluOpType

# token-columns per DMA chunk, per batch (last batch tapers for a short tail)
CHUNKS = [
    [4, 4, 4, 4],
    [4, 4, 4, 4],
    [4, 4, 4, 4],
    [4, 4, 4, 2, 1, 1],
]
NA = 12                    # split point for the last batch's softmax


def build_nc() -> bass.Bass:
    nc = bacc.Bacc("TRN2", target_bir_lowering=False, debug=False)
    st_d = nc.declare_dram_parameter("st", [BPC, D], F32, isOutput=False)
    hx_d = nc.declare_dram_parameter("hx", [BPC, S, D], F32R, isOutput=False)
    hm_d = nc.declare_dram_parameter("hx_mask", [BPC, S], F32, isOutput=False)
    w_d = nc.declare_dram_parameter("W", [D, D], F32, isOutput=False)
    bv_d = nc.declare_dram_parameter("b", [D], F32, isOutput=False)
    id_d = nc.declare_dram_parameter("ident", [P, P], F32, isOutput=False)
    ct_d = nc.declare_dram_parameter("ct", [BPC, D], F32, isOutput=True)

    with tile.TileContext(nc) as tc, ExitStack() as ctx:
        const = ctx.enter_context(tc.tile_pool(name="const", bufs=1))
        wtp = ctx.enter_context(tc.tile_pool(name="wtp", bufs=1))
        hxp = ctx.enter_context(tc.tile_pool(name="hxp", bufs=1))
        scrp = ctx.enter_context(tc.tile_pool(name="scrp", bufs=1))
        smp = ctx.enter_context(tc.tile_pool(name="smp", bufs=2))
        psp = ctx.enter_context(tc.tile_pool(name="psp", bufs=2, space="PSUM"))

        # ---- tiny inputs split across the two HWDGE rings ----
        ident = const.tile([P, P], F32, name="ident_sb")
        nc.sync.dma_start(out=ident[:, :], in_=id_d[:, :])
        # st_sb shares its slot with uA_sb/ct_rows (all temporally disjoint)
        st_sb = smp.tile([BPC, D], F32, name="st_sb", tag="big4", bufs=1)
        nc.scalar.dma_start(out=st_sb[:, :], in_=st_d[:, :])
        # bias replicated straight into its 4 output rows (no broadcast pass)
        bias4 = const.tile([BPC, D], F32, name="bias4")
        for b in range(BPC):
            nc.scalar.dma_start(out=bias4[b:b + 1, :], in_=bv_d[None, :])
        # mask straight into softmax layout: mask1[p, b*16+i] = hm[b, 16p+i]
        mask1 = const.tile([P, BPC * NT], F32, name="mask1")
        for b in range(BPC):
            nc.scalar.dma_start(
                out=mask1[:, b * NT:(b + 1) * NT],
                in_=hm_d[b, :].rearrange("(p i) -> p i", i=NT),
            )

        # W natural: wn[p, j*D+d] = W[8p+j, d], f16-cast over SWDGE Q0 --
        # runs concurrent with the two HWDGE hx rings and halves residency.
        wn = wtp.tile([P, WJ * D], F16, name="wn", tag="wn")
        for half in range(2):
            nc.gpsimd.dma_start(
                out=wn[:, half * (WJ // 2) * D:(half + 1) * (WJ // 2) * D]
                .rearrange("p (j d) -> p j d", d=D),
                in_=w_d.rearrange("(p jj j) d -> p jj j d", p=P, jj=2)[
                    :, half, :, :
                ],
            )
        # sacrificial trailing transfer: the last SWDGE transfer in a
        # near-dry queue drains at a crawl, so make it one nobody waits for
        junk = scrp.tile([1, HF], F16, name="junk", tag="junk")
        nc.gpsimd.dma_start(out=junk[0:1, :], in_=w_d[0:1, 0:HF])

        # ---- hx chunk DMAs: rolling 2-batch window, rings alternate ----
        hxbuf = [
            hxp.tile([P, NT * D], F32R, name=f"hxw{w}", tag=f"hxw{w}")
            for w in range(2)
        ]

        chunk_plan = []            # (b, c0 col, ncols, ring) in stream order
        ci = 0
        for b in range(BPC):
            c0 = 0
            for ncols in CHUNKS[b]:
                chunk_plan.append((b, c0, ncols, ci % 2))
                c0 += ncols
                ci += 1

        def emit_chunk(k):
            b, c0, ncols, ring = chunk_plan[k]
            eng = nc.sync if ring == 0 else nc.scalar
            buf = hxbuf[b % 2]
            # partition p <- tokens 16p+c0 .. 16p+c0+ncols-1 (contiguous rows)
            eng.dma_start(
                out=buf[:, c0 * D:(c0 + ncols) * D].rearrange(
                    "p (i d) -> p i d", d=D
                ),
                in_=hx_d[b, :, :].rearrange("(p i) d -> p i d", p=P)[
                    :, c0:c0 + ncols, :
                ],
            )

        NCH = len(chunk_plan)
        for k in range(8):         # b0 + b1 fill both rings + 8 sem lanes
            emit_chunk(k)
        next_chunk = [8]

        def emit_chunks(n):
            for _ in range(n):
                if next_chunk[0] < NCH:
                    emit_chunk(next_chunk[0])
                    next_chunk[0] += 1

        # ---- small consts ----
        ones_row = const.tile([1, P], F32, name="ones_row")
        nc.vector.memset(ones_row[:, :], 1.0)
        ones_row_r = const.tile([1, P], F32R, name="ones_row_r")
        nc.scalar.copy(ones_row_r[:, :], ones_row[:, :])
        mneg_row = const.tile([1, P], F32, name="mneg_row")
        nc.vector.memset(mneg_row[:, :], -1.0)
        ones_col = const.tile([P, 1], F32, name="ones_col")
        nc.vector.memset(ones_col[:, :], 1.0)
        ones_b = const.tile([1, BPC], F32, name="ones_b")
        nc.vector.memset(ones_b[:, :], 1.0)

        ident_h = const.tile([P, P], F16, name="ident_h")
        nc.scalar.copy(ident_h[:, :], ident[:, :])

        # ---- stT2[p, j*4+b] = st[b, 8p+j] (strided transposes, f16) ----
        stT = const.tile([P, WJ * BPC], F16, name="stT")
        st_v = st_sb[0:BPC, :].rearrange("b (p j) -> b p j", j=WJ)
        for j in range(WJ):
            tp = psp.tile([P, P], F32, name=f"tp_st{j}", tag="tr")
            nc.tensor.transpose(
                tp[:, 0:BPC], st_v[:, :, j], ident[0:BPC, 0:BPC]
            )
            nc.scalar.copy(stT[:, j * BPC:(j + 1) * BPC], tp[:, 0:BPC])

        # ---- v = st@W: v[b,e] = sum_j sum_p st[b,8p+j] W[8p+j,e] ----
        v_ps = [
            psp.tile([BPC, HF], F32, name=f"v_ps{h}", tag=f"mm{h}", bufs=1)
            for h in range(2)
        ]
        for j in range(WJ):
            lt = stT[:, j * BPC:(j + 1) * BPC]
            for h in range(2):
                nc.tensor.matmul(
                    v_ps[h][:, :], lt,
                    wn[:, j * D + h * HF:j * D + (h + 1) * HF],
                    start=(j == 0), stop=(j == WJ - 1),
                )
        v_sb = const.tile([BPC, D], F32R, name="v_sb")
        for h in range(2):
            nc.scalar.copy(v_sb[:, h * HF:(h + 1) * HF], v_ps[h][:, :])

        # ---- vb[b] = broadcast of v row b to 128 partitions ----
        vb_tiles = []
        for b in range(BPC):
            vb = const.tile([P, D], F32, name=f"vb{b}")
            if b == 0:
                v_row = v_sb
            else:
                v_row = smp.tile([1, D], F32R, name=f"v_row{b}", tag="u_sb", bufs=1)
                nc.scalar.dma_start(out=v_row[:, :], in_=v_sb[b:b + 1, :])
            for h in range(2):
                bc = psp.tile([P, HF], F32, name=f"vbc{b}_{h}", tag=f"mm{h}", bufs=1)
                nc.tensor.matmul(
                    bc[:, :], ones_row_r[:, 0:P],
                    v_row[0:1, h * HF:(h + 1) * HF],
                    start=True, stop=True,
                )
                nc.scalar.copy(vb[:, h * HF:(h + 1) * HF], bc[:, :])
            vb_tiles.append(vb)

        # ---- W^T tiles (f16; only needed for the final ct matmuls) ----
        # wt[k][dp, e] = W[e, k*128+dp]; e = 8p+j -> strided column writes
        wt_tiles = [
            wtp.tile([P, D], F16, name=f"wt{k}", tag=f"wt{k}") for k in range(DCH)
        ]

        def emit_wt(j_list):
            for j in j_list:
                for i in range(DCH):
                    tp = psp.tile([P, P], F16, name=f"tp_w{j}_{i}", tag="trh")
                    nc.tensor.transpose(
                        tp[:, :], wn[:, j * D + i * P:j * D + (i + 1) * P],
                        ident_h[:, :],
                    )
                    # tp[dp, p] = W[8p+j, i*128+dp] -> wt[i][:, j::8]
                    nc.scalar.copy(
                        wt_tiles[i][:, :].rearrange(
                            "q (p j) -> q p j", j=WJ
                        )[:, :, j],
                        tp[:, :],
                    )

        # ---- per-batch streaming machinery ----
        ut_tiles = [
            const.tile([P, BPC], F16, name=f"ut{k}") for k in range(DCH)
        ]
        l_all = const.tile([P, BPC], F32, name="l_all")
        scr = scrp.tile([P, D], F16, name="scr", tag="scr")

        def emit_scores(b, i_lo, i_hi, score):
            buf = hxbuf[b % 2]
            for i in range(i_lo, i_hi):
                nc.vector.scalar_tensor_tensor(
                    out=scr[:, :],
                    in0=buf[:, i * D:(i + 1) * D].bitcast(F32),
                    scalar=1.0,
                    in1=vb_tiles[b][:, :],
                    op0=ALU.mult,
                    op1=ALU.mult,
                    accum_out=score[:, i:i + 1],
                )

        def neg_max(score_m, cols, nm_name):
            """global max over score_m[:, :cols] -> Mx [1,1]"""
            m1 = smp.tile([P, 1], F32, name=f"m1_{nm_name}", tag="m1")
            nc.vector.tensor_reduce(
                m1[:, :], score_m[:, 0:cols], mybir.AxisListType.X, ALU.max
            )
            tp_m = psp.tile([1, P], F32, name=f"tpm_{nm_name}", tag="tr")
            nc.tensor.transpose(tp_m[0:1, :], m1[:, 0:1], ident[:, :])
            Mx = smp.tile([1, 1], F32, name=f"Mx_{nm_name}", tag="Mx")
            nc.vector.tensor_reduce(
                Mx[:, :], tp_m[0:1, :], mybir.AxisListType.X, ALU.max
            )
            return Mx

        def bcast_neg(Mx, nm_name):
            bc_m = psp.tile([P, 1], F32, name=f"bcm_{nm_name}", tag="tr")
            nc.tensor.matmul(
                bc_m[:, :], mneg_row[:, :], Mx[:, :],
                start=True, stop=True,
            )
            negm_b = smp.tile([P, 1], F32, name=f"negmb_{nm_name}", tag="negmb")
            nc.scalar.copy(negm_b[:, :], bc_m[:, :])
            return negm_b

        def emit_u(b, i_lo, i_hi, e_sb, e_off, u_ps):
            buf = hxbuf[b % 2]
            for i in range(i_lo, i_hi):
                for hf in range(2):
                    nc.tensor.matmul(
                        u_ps[hf][:, :],
                        e_sb[:, i - e_off:i - e_off + 1],
                        buf[:, i * D + hf * HF:i * D + (hf + 1) * HF],
                        start=(i == i_lo), stop=(i == i_hi - 1),
                    )

        def emit_ut(b, u_sb):
            for k in range(DCH):
                tp_u = psp.tile([P, 1], F32, name=f"tpu_{b}_{k}", tag="tr")
                nc.tensor.transpose(
                    tp_u[:, 0:1], u_sb[0:1, k * P:(k + 1) * P], ident[0:1, 0:1]
                )
                nc.scalar.copy(ut_tiles[k][:, b:b + 1], tp_u[:, 0:1])

        def emit_post(b, score):
            """softmax chain + u + ut for a completed batch's scores"""
            score_m = smp.tile([P, NT], F32, name=f"score_m{b}", tag="score_m")
            nc.vector.scalar_tensor_tensor(
                out=score_m[:, :], in0=score[:, :], scalar=1.0,
                in1=mask1[:, b * NT:(b + 1) * NT], op0=ALU.mult, op1=ALU.mult,
            )
            negm_b = bcast_neg(neg_max(score_m, NT, f"b{b}"), f"b{b}")
            e_sb = smp.tile([P, NT], F32R, name=f"e{b}", tag="e")
            nc.scalar.activation(
                e_sb[:, :], score_m[:, :], AF.Exp,
                bias=negm_b[:, 0:1], scale=1.0, accum_out=l_all[:, b:b + 1],
            )
            u_ps = [
                psp.tile([1, HF], F32, name=f"u_ps{b}_{h}", tag=f"mm{h}", bufs=1)
                for h in range(2)
            ]
            emit_u(b, 0, NT, e_sb, 0, u_ps)
            u_sb = smp.tile([1, D], F32, name=f"u_sb{b}", tag="u_sb", bufs=1)
            for hf in range(2):
                nc.scalar.copy(u_sb[:, hf * HF:(hf + 1) * HF], u_ps[hf][:, :])
            emit_ut(b, u_sb)

        # ---- batch 0 ----
        # NOTE: batch b+2's chunk DMAs reuse hxbuf[b%2], so they are traced
        # only after every reader of batch b (scores AND u matmuls); Tile
        # then serializes them WAR-correctly.
        score0 = smp.tile([P, NT], F32, name="score0", tag="score")
        emit_scores(0, 0, NT, score0)
        emit_post(0, score0)
        emit_chunks(4)           # b2 chunks (overwrite b0's buffer)

        # ---- batch 1 ----
        score1 = smp.tile([P, NT], F32, name="score1", tag="score")
        emit_scores(1, 0, NT, score1)
        emit_post(1, score1)
        emit_chunks(4)           # b3 chunks 0-3 (overwrite b1's buffer)

        # ---- batch 2 ----
        score2 = smp.tile([P, NT], F32, name="score2", tag="score")
        emit_scores(2, 0, NT, score2)
        emit_post(2, score2)
        emit_chunks(2)           # b3 taper chunks
        emit_wt(range(WJ))

        # ---- batch 3: two-stage softmax for a short tail ----
        b = BPC - 1
        score3 = smp.tile([P, NT], F32, name="score3", tag="score")
        emit_scores(b, 0, NA, score3)

        # stage A over cols [0, NA)
        score_mA = smp.tile([P, NA], F32, name="score_mA", tag="score_m")
        nc.vector.scalar_tensor_tensor(
            out=score_mA[:, :], in0=score3[:, 0:NA], scalar=1.0,
            in1=mask1[:, b * NT:b * NT + NA], op0=ALU.mult, op1=ALU.mult,
        )
        MxA = neg_max(score_mA, NA, "A")
        negm_A = bcast_neg(MxA, "A")
        e_A = smp.tile([P, NA], F32R, name="e_A", tag="e")
        l1_A = smp.tile([P, 1], F32, name="l1_A", tag="l1A")
        nc.scalar.activation(
            e_A[:, :], score_mA[:, :], AF.Exp,
            bias=negm_A[:, 0:1], scale=1.0, accum_out=l1_A[:, 0:1],
        )
        uA_ps = [
            psp.tile([1, HF], F32, name=f"uA_ps{h}", tag=f"mm{h}", bufs=1)
            for h in range(2)
        ]
        emit_u(b, 0, NA, e_A, 0, uA_ps)

        emit_scores(b, NA, NT, score3)

        # stage B over cols [NA, NT) with the true global max
        NB = NT - NA
        score_mB = smp.tile([P, NB], F32, name="score_mB", tag="score_mB")
        nc.vector.scalar_tensor_tensor(
            out=score_mB[:, :], in0=score3[:, NA:NT], scalar=1.0,
            in1=mask1[:, b * NT + NA:(b + 1) * NT], op0=ALU.mult, op1=ALU.mult,
        )
        MxB = neg_max(score_mB, NB, "B")
        Mf = smp.tile([1, 1], F32, name="Mf", tag="Mf")
        nc.vector.tensor_scalar_max(Mf[:, :], MxB[:, :], MxA[0:1, 0:1])
        negm_f = bcast_neg(Mf, "F")
        e_B = smp.tile([P, NB], F32R, name="e_B", tag="eB")
        l1_B = smp.tile([P, 1], F32, name="l1_B", tag="l1B")
        nc.scalar.activation(
            e_B[:, :], score_mB[:, :], AF.Exp,
            bias=negm_f[:, 0:1], scale=1.0, accum_out=l1_B[:, 0:1],
        )
        # sA = exp(MxA - Mf), scalar and [128,1] broadcast forms
        dA = smp.tile([1, 1], F32, name="dA", tag="dA")
        nc.vector.tensor_scalar_add(dA[:, :], MxA[:, :], negm_f[0:1, 0:1])
        sA = smp.tile([1, 1], F32, name="sA", tag="sA")
        nc.scalar.activation(sA[:, :], dA[:, :], AF.Exp, bias=0.0, scale=1.0)
        sA_bc = psp.tile([P, 1], F32, name="sA_bc", tag="tr")
        nc.tensor.matmul(
            sA_bc[:, :], ones_row[:, :], sA[:, :],
            start=True, stop=True,
        )
        sA_b = smp.tile([P, 1], F32, name="sA_b", tag="sAb")
        nc.scalar.copy(sA_b[:, :], sA_bc[:, :])
        # l_all[:, 3] = l1_A * sA + l1_B
        nc.vector.scalar_tensor_tensor(
            out=l_all[:, b:b + 1], in0=l1_A[:, 0:1], scalar=sA_b[:, 0:1],
            in1=l1_B[:, 0:1], op0=ALU.mult, op1=ALU.add,
        )
        # u_A scaled by sA while B streams in
        uA_sb = smp.tile([1, D], F32, name="uA_sb", tag="big4", bufs=1)
        for hf in range(2):
            nc.scalar.mul(
                uA_sb[:, hf * HF:(hf + 1) * HF], uA_ps[hf][:, :],
                mul=sA[0:1, 0:1],
            )
        uB_ps = [
            psp.tile([1, HF], F32, name=f"uB_ps{h}", tag=f"mm{h}", bufs=1)
            for h in range(2)
        ]
        emit_u(b, NA, NT, e_B, NA, uB_ps)
        u_sb = smp.tile([1, D], F32, name=f"u_sb{b}", tag="u_sb", bufs=1)
        for hf in range(2):
            nc.vector.scalar_tensor_tensor(
                out=u_sb[:, hf * HF:(hf + 1) * HF], in0=uB_ps[hf][:, :],
                scalar=1.0, in1=uA_sb[:, hf * HF:(hf + 1) * HF],
                op0=ALU.mult, op1=ALU.add,
            )
        emit_ut(b, u_sb)

        # ---- L reciprocal for all batches ----
        L_ps = psp.tile([BPC, 1], F32, name="L_ps", tag="tr")
        nc.tensor.matmul(
            L_ps[:, :], l_all[:, 0:BPC], ones_col[:, 0:1],
            start=True, stop=True,
        )
        recip4 = smp.tile([BPC, 1], F32, name="recip4", tag="recip4")
        nc.vector.reciprocal(recip4[:, :], L_ps[0:BPC, 0:1])

        # ---- ct = (u_raw @ W^T) * (1/L) + b for all 4 batches ----
        ct_rows = smp.tile([BPC, D], F32, name="ct_rows", tag="big4", bufs=1)
        for hf in range(2):
            ctp = psp.tile([BPC, HF], F32, name=f"ct_ps{hf}", tag=f"mm{hf}", bufs=1)
            for k in range(DCH):
                nc.tensor.matmul(
                    ctp[:, :], ut_tiles[k][:, 0:BPC],
                    wt_tiles[k][:, hf * HF:(hf + 1) * HF],
                    start=(k == 0), stop=(k == DCH - 1),
                )
            nc.vector.scalar_tensor_tensor(
                out=ct_rows[:, hf * HF:(hf + 1) * HF], in0=ctp[:, :],
                scalar=recip4[0:BPC, 0:1],
                in1=bias4[:, hf * HF:(hf + 1) * HF],
                op0=ALU.mult, op1=ALU.add,
            )
        nc.sync.dma_start(out=ct_d[:, :], in_=ct_rows[:, :])

    nc.compile()
    return nc


_NC_CACHE = {}


def get_nc() -> bass.Bass:
    if "nc" not in _NC_CACHE:
        _NC_CACHE["nc"] = build_nc()
    return _NC_CACHE["nc"]


def kernel(st, hx, hx_mask, W, b):
    nc = get_nc()
    ident = np.eye(P, dtype=np.float32)
    in_maps = []
    for i in range(NCORES):
        sl = slice(i * BPC, (i + 1) * BPC)
        in_maps.append(
            {
                "st": np.ascontiguousarray(st[sl], dtype=np.float32),
                "hx": np.ascontiguousarray(hx[sl], dtype=np.float32),
                "hx_mask": np.ascontiguousarray(hx_mask[sl], dtype=np.float32),
                "W": np.asarray(W, dtype=np.float32),
                "b": np.asarray(b, dtype=np.float32),
                "ident": ident,
            }
        )
    res = run_bass_kernel_spmd(nc, in_maps, list(range(NCORES)))
    out = np.concatenate([res.results[i]["ct"] for i in range(NCORES)], axis=0)
    return out.astype(np.float32)
